# revision 14
# baseline (speedup 1.0000x reference)
"""DETR-style matcher cost matrix on 8 Trainium2 NeuronCores.

cost[b, g, p] = -pred_cls[b, p, g]
                + mean(|pred_box[p] - gt_box[g]|)          (L1, 4 coords)
                + 1 - IoU + (area_c - union)/(area_c+eps)  (GIoU loss)
masked to zero where gt_validity[b, g] == 0.

Sharding: data-parallel over batch, 4 batches per core (B=32, 8 cores).

Layout per (batch, gt-tile of 128): [128 part = gt rows, 900 free = preds].
Per-pred values enter as partition-broadcast maps (fp16 for 2x DVE modes),
per-gt values as [128,1] fp32 scalars.  Identities used:
  wi0   = min(Px2,Gx2) - max(Px1,Gx1)        pre-relu intersection width
  wc    = (wp + wg) - wi0                    enclosing-box width
  l1*4  = (wc + hc) - (wi0 + hi0) = (wp+wg+hp+hg) - 2*(wi0+hi0)
  inter = relu(wi0)*relu(hi0)
  union = area_p + area_g - inter
  t2    = (area_c - union)/(area_c) ~= 1 - union/area_c   (eps folded)
  cost  = V*(0.25*SWH - 0.5*s2 + 2 - iou - union/area_c) - V*clsT
The fp32 division tail uses RECIPROCAL_APPROX_FAST (~51 ULP).
pred_cls.T comes via PE transposes into PSUM; ScalarE folds it to
V*(2 - clsT) in SBUF so the final combine is one scalar_tensor_tensor.
"""

import numpy as np

B, Q = 32, 900
N_CORES = 8
B_PER = B // N_CORES
EPS = 1e-7
GT = 8  # gt tiles per batch: 7 full x128 + 1 of 4 rows
PT = 8  # pred chunks of 128 (last = 4)

USE_CUSTOM = True  # authored fused DVE ops (W0_IOU_ANT / RELUMUL_ANT)
USE_GP = False  # offload some fp32 tail ops to GpSimd

_cached = {}


def _split_multi_waits(nc):
    """This neuronxcc build rejects >1 sync-wait per instruction. Split any
    instruction carrying N>1 waits by inserting N-1 wait-carrier nops before
    it on the same (in-order) engine stream."""
    import concourse.mybir as mybir

    for fn in nc.m.functions:
        for bb in fn.blocks:
            out = []
            for ins in bb.instructions:
                si = getattr(ins, "sync_info", None)
                waits = list(si.on_wait) if (si and si.on_wait) else []
                if len(waits) > 1:
                    si.on_wait = [waits[-1]]
                    for j, w in enumerate(waits[:-1]):
                        nop = mybir.InstNoOp(name=f"{ins.name}-sw{j}", ins=[], outs=[])
                        nop.engine = ins.engine
                        nop.sync_info = mybir.SyncInfo(on_wait=[w], on_update=[])
                        out.append(nop)
                out.append(ins)
            bb.instructions[:] = out


def _ensure_custom_ops():
    """Author two fused DVE ops and register them in dve_ops' tables:
      W0_IOU_ANT:  out = min(in0, s0) - max(in1, s1)
      RELUMUL_ANT: out = relu(in0) * relu(in1)
    """
    from concourse import dve_ops
    from concourse.dve_spec import Spec, Src0, Src1, C0, C1, minn, maxx, relu
    from concourse.dve_spec import lower, _has_src1
    from concourse.dve_uop import DveOpSpec

    if "W0_IOU_ANT" in dve_ops._SUB_OPCODE_FOR_NAME:
        return

    from concourse.dve_spec import C2

    def author(name, body, ref):
        spec = Spec(body=body, reference=ref)
        row = max(dve_ops._SUB_OPCODE_FOR_NAME.values()) + 1
        shas = {}
        for ver in ("v3", "v4"):
            uops = lower(spec, ver=ver)
            s = DveOpSpec(name=name, opcode=row, uops=uops, rd1_en=_has_src1(spec))
            shas[ver] = s.sha(ver)
        op = dve_ops.DveOp(name, spec, False, shas)
        dve_ops.OPS.append(op)
        dve_ops.CUSTOM_DVE_SPECS[name] = spec
        dve_ops._SUB_OPCODE_FOR_NAME[name] = row
        return op

    w0 = author(
        "W0_IOU_ANT",
        (minn(Src0, C0) - maxx(Src1, C1)) * C2,
        lambda in0, in1, s0, s1, imm2: (np.minimum(in0, s0) - np.maximum(in1, s1))
        * imm2,
    )
    rm = author(
        "RELUMUL_ANT",
        relu(Src0) * relu(Src1) * C2,
        lambda in0, in1, s0, s1, imm2: np.maximum(in0, 0.0)
        * np.maximum(in1, 0.0)
        * imm2,
    )
    return w0, rm


def _by_name(dve_ops, name):
    for op in dve_ops.OPS:
        if op.name == name:
            return op
    raise KeyError(name)


def _build_nc():
    import concourse.bass as bass
    from concourse import mybir, dve_ops
    from concourse.tile import TileContext
    from concourse.masks import make_identity

    if USE_CUSTOM:
        _ensure_custom_ops()
        W0 = _by_name(dve_ops, "W0_IOU_ANT")
        RM = _by_name(dve_ops, "RELUMUL_ANT")

    f32 = mybir.dt.float32
    f16 = mybir.dt.float16
    Alu = mybir.AluOpType
    Act = mybir.ActivationFunctionType

    nc = bass.Bass()
    pb_d = nc.dram_tensor("pred_boxes", [B_PER, Q, 4], f32, kind="ExternalInput")
    gb_d = nc.dram_tensor("gt_boxes", [B_PER, Q, 4], f32, kind="ExternalInput")
    cls_d = nc.dram_tensor("pred_cls", [B_PER, Q, Q], f32, kind="ExternalInput")
    val_d = nc.dram_tensor("validity", [B_PER, Q], f32, kind="ExternalInput")
    cost_d = nc.dram_tensor("cost", [B_PER, Q, Q], f32, kind="ExternalOutput")

    with TileContext(nc) as tc:
        with (
            tc.tile_pool(name="const", bufs=1) as constp,
            tc.tile_pool(name="batch", bufs=2) as batchp,
            tc.tile_pool(name="cls", bufs=3) as clsp,
            tc.tile_pool(name="chain", bufs=2) as chp,
            tc.tile_pool(name="outp", bufs=3) as outp,
            tc.tile_pool(name="psum", bufs=2, space="PSUM") as psp,
        ):
            ident = constp.tile([128, 128], f32)
            make_identity(nc, ident)

            gp = nc.gpsimd if USE_GP else nc.vector

            # widths are carried scaled by SC=256 in fp16 to stay clear of
            # fp16 subnormals; SC folds back out via imm scalars downstream.
            SC = 256.0 if USE_CUSTOM else 1.0
            ISC2 = 1.0 / (SC * SC)
            hdt = f16 if USE_CUSTOM else f32

            for b in range(B_PER):
                # ---- per-batch: pred maps ----------------------------------
                # map4[part, 4*p + c] = pred_boxes[b, p, c] broadcast to all parts
                map4 = batchp.tile([128, 4 * Q], f32, tag="map4")
                src = pb_d[b][:].flatten()  # [3600]
                bcast = bass.AP(
                    tensor=src.tensor, offset=src.offset, ap=[[0, 128]] + list(src.ap)
                )
                nc.sync.dma_start(out=map4[:], in_=bcast)
                m4 = map4[:].rearrange("p (q c) -> p c q", c=4)
                m4x1, m4y1, m4x2, m4y2 = (m4[:, c, :] for c in range(4))

                # WPh = SC*(x2-x1) etc., computed from fp32 coords (no
                # catastrophic cancellation), stored fp16
                WPh = batchp.tile([128, Q], hdt, tag="WPh")
                HPh = batchp.tile([128, Q], hdt, tag="HPh")
                if USE_CUSTOM:
                    nc.vector._custom_dve(
                        W0, out=WPh[:], in0=m4x2, in1=m4x1, s0=1e30, s1=-1e30, imm2=SC
                    )
                    nc.vector._custom_dve(
                        W0, out=HPh[:], in0=m4y2, in1=m4y1, s0=1e30, s1=-1e30, imm2=SC
                    )
                else:
                    nc.vector.tensor_sub(WPh[:], m4x2, m4x1)
                    nc.vector.tensor_sub(HPh[:], m4y2, m4y1)
                SPs = batchp.tile([128, Q], hdt, tag="SPs")
                nc.vector.tensor_add(SPs[:], WPh[:], HPh[:])
                SPh4 = batchp.tile([128, Q], hdt, tag="SPh4")
                nc.vector.tensor_scalar_mul(SPh4[:], SPs[:], 0.25 / SC)
                APm = batchp.tile([128, Q], f32, tag="APm")
                if USE_CUSTOM:
                    nc.vector._custom_dve(
                        RM, out=APm[:], in0=WPh[:], in1=HPh[:], imm2=ISC2
                    )
                else:
                    nc.vector.tensor_mul(APm[:], WPh[:], HPh[:])

                # ---- per-batch: gt scalars ---------------------------------
                gall = batchp.tile([128, GT, 4], f32, tag="gall")
                nc.gpsimd.memset(gall[:], 0.5)
                nc.sync.dma_start(
                    out=gall[:, 0:7, :],
                    in_=gb_d[b, 0:896, :].rearrange("(t p) c -> p t c", p=128),
                )
                nc.sync.dma_start(out=gall[0:4, 7, :], in_=gb_d[b, 896:900, :])

                vall = batchp.tile([128, GT], f32, tag="vall")
                nc.gpsimd.memset(vall[:], 0.0)
                nc.sync.dma_start(
                    out=vall[:, 0:7],
                    in_=val_d[b, 0:896].rearrange("(t p) -> p t", p=128),
                )
                nc.sync.dma_start(out=vall[0:4, 7], in_=val_d[b, 896:900])

                WG = batchp.tile([128, GT], f32, tag="WG")
                nc.vector.tensor_sub(WG[:], gall[:, :, 2], gall[:, :, 0])
                HG = batchp.tile([128, GT], f32, tag="HG")
                nc.vector.tensor_sub(HG[:], gall[:, :, 3], gall[:, :, 1])
                AGe = batchp.tile([128, GT], f32, tag="AGe")
                nc.vector.tensor_mul(AGe[:], WG[:], HG[:])
                nc.vector.tensor_scalar_add(AGe[:], AGe[:], float(EPS))
                SG4 = batchp.tile([128, GT], f32, tag="SG4")
                nc.vector.tensor_add(SG4[:], WG[:], HG[:])
                nc.vector.tensor_scalar_mul(SG4[:], SG4[:], 0.25)
                WGs = batchp.tile([128, GT], f32, tag="WGs")
                nc.vector.tensor_scalar_mul(WGs[:], WG[:], SC)
                HGs = batchp.tile([128, GT], f32, tag="HGs")
                nc.vector.tensor_scalar_mul(HGs[:], HG[:], SC)
                negV = batchp.tile([128, GT], f32, tag="negV")
                nc.vector.tensor_scalar_mul(negV[:], vall[:], -1.0)
                twoV = batchp.tile([128, GT], f32, tag="twoV")
                nc.vector.tensor_scalar_mul(twoV[:], vall[:], 2.0)

                # ---- per gt-tile unit --------------------------------------
                for t in range(GT):
                    g0 = t * 128
                    gw = 128 if t < 7 else 4

                    # cls chunks in; PE-transpose into PSUM: ps[g,p] = cls[p,g]
                    clsin = clsp.tile([128, PT, 128], f32, tag="clsin")
                    for k in range(PT):
                        p0 = k * 128
                        pw = 128 if k < 7 else 4
                        nc.sync.dma_start(
                            out=clsin[0:pw, k, 0:gw],
                            in_=cls_d[b, p0 : p0 + pw, g0 : g0 + gw],
                        )
                    psA = psp.tile([128, 512], f32, tag="psA")
                    psB = psp.tile([128, 388], f32, tag="psB")
                    for k in range(PT):
                        p0 = k * 128
                        pw = 128 if k < 7 else 4
                        dst = (
                            psA[0:gw, p0 : p0 + pw]
                            if p0 < 512
                            else psB[0:gw, p0 - 512 : p0 - 512 + pw]
                        )
                        nc.tensor.transpose(dst, clsin[0:pw, k, 0:gw], ident[0:pw, 0:pw])

                    # clsV = V*(2 - clsT), folded on ScalarE from PSUM
                    Vt = vall[:, t : t + 1]
                    negVt = negV[:, t : t + 1]
                    twoVt = twoV[:, t : t + 1]
                    clsV = chp.tile([128, Q], f32, tag="clsV")
                    nc.scalar.activation(
                        clsV[:, 0:512], psA[:, :], Act.Identity, bias=twoVt, scale=negVt
                    )
                    nc.scalar.activation(
                        clsV[:, 512:900], psB[:, :], Act.Identity, bias=twoVt, scale=negVt
                    )

                    Gx1 = gall[:, t, 0:1]
                    Gy1 = gall[:, t, 1:2]
                    Gx2 = gall[:, t, 2:3]
                    Gy2 = gall[:, t, 3:4]
                    WGst = WGs[:, t : t + 1]
                    HGst = HGs[:, t : t + 1]
                    AGet = AGe[:, t : t + 1]
                    SG4t = SG4[:, t : t + 1]

                    stt = nc.vector.scalar_tensor_tensor

                    # geometry head: widths scaled by SC, stored fp16,
                    # computed from fp32 coords
                    wi0 = chp.tile([128, Q], hdt, tag="wi0")
                    hi0 = chp.tile([128, Q], hdt, tag="hi0")
                    if USE_CUSTOM:
                        nc.vector._custom_dve(
                            W0, out=wi0[:], in0=m4x2, in1=m4x1, s0=Gx2, s1=Gx1, imm2=SC
                        )
                        nc.vector._custom_dve(
                            W0, out=hi0[:], in0=m4y2, in1=m4y1, s0=Gy2, s1=Gy1, imm2=SC
                        )
                    else:
                        Mx1 = chp.tile([128, Q], f32, tag="Mx1")
                        nc.vector.tensor_scalar_max(Mx1[:], m4x1, Gx1)
                        mx2 = chp.tile([128, Q], f32, tag="mx2")
                        nc.vector.tensor_scalar_min(mx2[:], m4x2, Gx2)
                        nc.vector.tensor_sub(wi0[:], mx2[:], Mx1[:])
                        My1 = chp.tile([128, Q], f32, tag="My1")
                        nc.vector.tensor_scalar_max(My1[:], m4y1, Gy1)
                        my2 = chp.tile([128, Q], f32, tag="my2")
                        nc.vector.tensor_scalar_min(my2[:], m4y2, Gy2)
                        nc.vector.tensor_sub(hi0[:], my2[:], My1[:])

                    s2 = chp.tile([128, Q], hdt, tag="s2")
                    nc.vector.tensor_add(s2[:], wi0[:], hi0[:])

                    W = chp.tile([128, Q], hdt, tag="W")
                    nc.vector.tensor_scalar_add(W[:], WPh[:], WGst)
                    wc = chp.tile([128, Q], hdt, tag="wc")
                    nc.vector.tensor_sub(wc[:], W[:], wi0[:])
                    H = chp.tile([128, Q], hdt, tag="H")
                    nc.vector.tensor_scalar_add(H[:], HPh[:], HGst)
                    hc = chp.tile([128, Q], hdt, tag="hc")
                    nc.vector.tensor_sub(hc[:], H[:], hi0[:])

                    # fp32 tail
                    inter = chp.tile([128, Q], f32, tag="inter")
                    areac = chp.tile([128, Q], f32, tag="areac")
                    if USE_CUSTOM:
                        nc.vector._custom_dve(
                            RM, out=inter[:], in0=wi0[:], in1=hi0[:], imm2=ISC2
                        )
                        nc.vector._custom_dve(
                            RM, out=areac[:], in0=wc[:], in1=hc[:], imm2=ISC2
                        )
                    else:
                        wiR = chp.tile([128, Q], f32, tag="wiR")
                        nc.vector.tensor_scalar_max(wiR[:], wi0[:], 0.0)
                        hiR = chp.tile([128, Q], f32, tag="hiR")
                        nc.vector.tensor_scalar_max(hiR[:], hi0[:], 0.0)
                        nc.vector.tensor_mul(inter[:], wiR[:], hiR[:])
                        nc.vector.tensor_mul(areac[:], wc[:], hc[:])
                    union = chp.tile([128, Q], f32, tag="union")
                    stt(union[:], APm[:], AGet, inter[:], Alu.add, Alu.subtract)

                    rcu = chp.tile([128, Q], f32, tag="rcu")
                    nc.vector.reciprocal_approx_fast(out=rcu[:], in_=union[:])
                    rca = chp.tile([128, Q], f32, tag="rca")
                    nc.vector.reciprocal_approx_fast(out=rca[:], in_=areac[:])

                    u1 = chp.tile([128, Q], f32, tag="u1")
                    gp.tensor_mul(u1[:], inter[:], rcu[:])
                    t2m = chp.tile([128, Q], f32, tag="t2m")
                    nc.vector.tensor_mul(t2m[:], union[:], rca[:])
                    c1 = chp.tile([128, Q], f32, tag="c1")
                    gp.tensor_add(c1[:], u1[:], t2m[:])

                    SWH4 = chp.tile([128, Q], hdt, tag="SWH4")
                    nc.vector.tensor_scalar_add(SWH4[:], SPh4[:], SG4t)
                    c3 = chp.tile([128, Q], f32, tag="c3")
                    stt(c3[:], s2[:], -0.5 / SC, SWH4[:], Alu.mult, Alu.add)
                    c4 = chp.tile([128, Q], f32, tag="c4")
                    nc.vector.tensor_sub(c4[:], c3[:], c1[:])

                    out = outp.tile([128, Q], f32, tag="out")
                    stt(out[:], c4[:], Vt, clsV[:], Alu.mult, Alu.add)

                    nc.sync.dma_start(
                        out=cost_d[b, g0 : g0 + gw, :], in_=out[0:gw, :]
                    )
    mybir.codegen_inst_isa_subclasses(nc)  # fill ISA bytes for custom-DVE ops
    _split_multi_waits(nc)
    return nc


def _get_nc():
    if "nc" not in _cached:
        _cached["nc"] = _build_nc()
    return _cached["nc"]


def _in_maps(pred_boxes, pred_cls, gt_boxes, gt_validity):
    maps = []
    for c in range(N_CORES):
        sl = slice(c * B_PER, (c + 1) * B_PER)
        maps.append(
            {
                "pred_boxes": np.ascontiguousarray(pred_boxes[sl], dtype=np.float32),
                "gt_boxes": np.ascontiguousarray(gt_boxes[sl], dtype=np.float32),
                "pred_cls": np.ascontiguousarray(pred_cls[sl], dtype=np.float32),
                "validity": np.ascontiguousarray(
                    gt_validity[sl].astype(np.float32)
                ),
            }
        )
    return maps


def kernel(pred_boxes, pred_cls, gt_boxes, gt_validity, _trace=False):
    from concourse import bass_utils

    nc = _get_nc()
    maps = _in_maps(pred_boxes, pred_cls, gt_boxes, gt_validity)
    res = bass_utils.run_bass_kernel_spmd(
        nc, maps, core_ids=list(range(N_CORES)), trace=_trace
    )
    out = np.concatenate([res.results[c]["cost"] for c in range(N_CORES)], axis=0)
    if _trace:
        _cached["last_result"] = res
    return out
